# revision 3
# baseline (speedup 1.0000x reference)
"""FBCritic embedding-lookup kernel for 8 Trainium2 NeuronCores.

Math (reference):
    fwd_idx = clip(obs)*10 + clip(act)            # [8192]
    bwd_idx = clip(fobs)*10 + clip(fact)          # [8192]
    F = W_f[fwd_idx]                              # [8192, 64]
    B = W_b[bwd_idx]                              # [8192, 64]
    out = F @ B.T                                 # [8192, 8192] f32

Sharding: 2D grid over the output — 4 row blocks x 2 col blocks. Core
c = a*2 + b computes out[a*2048:(a+1)*2048, b*4096:(b+1)*4096]. The 2D
split minimizes the number of indirect-gather instructions per core
(16 fwd + 32 bwd row-groups of 128) since the hardware SWDGE consumes
exactly one index per destination partition per indirect DMA, and each
descriptor-generation costs ~1us serialized on the GpSimd Q7.

Precision: tables are converted to f16 on the host (output rel err
~5e-4, far under the 2e-2 gate), the device computes in f16 with f32
PSUM accumulation, and the output is written as f16 then upcast on the
host. This halves both gather and output HBM traffic vs f32.

Per-core pipeline: per 128-row group one indirect DMA gathers [128, 64]
f16 rows; PE transposes groups into [64, 128] operand layout via an f16
identity matmul (f16 PSUM); DVE copies (2x mode) assemble fwdT
[64, 2048] / bwdT [64, 4096] in SBUF; matmuls [64,128]^T @ [64,512]
accumulate f32 PSUM strips; strip copies PSUM f32 -> SBUF f16 are split
between DVE and ACT; output DMAs [128, 1024] f16 strips go out on the
sync (SP) HWDGE queue. Gather/transpose units for later column chunks
are interleaved between matmul batches so every engine's in-order
stream stays busy.
"""

import numpy as np

NUM_OBS = 100000
NUM_ACT = 10
V = NUM_OBS * NUM_ACT  # 1_000_000 table rows
D = 64                 # repr dim
B = 8192               # batch
N_CORES = 8
RA = 4                 # row blocks
CB = 2                 # col blocks
MLOC = B // RA         # 2048 output rows per core
NLOC = B // CB         # 4096 output cols per core
P = 128                # partitions

GF = MLOC // P         # 16 forward 128-row groups
GB = NLOC // P         # 32 backward 128-row groups
NJ = 512               # matmul moving free dim (one PSUM bank)
JP = 1024              # strip width / transpose-psum width

_CACHE = {}


def _build_nc():
    import concourse.bass as bass
    import concourse.tile as tile
    from concourse import bacc, mybir
    from concourse.masks import make_identity

    f16 = mybir.dt.float16
    f32 = mybir.dt.float32
    i32 = mybir.dt.int32

    nc = bacc.Bacc("TRN2", target_bir_lowering=False, debug=False)

    wf = nc.dram_tensor("wf", [V, D], f16, kind="ExternalInput").ap()
    wb = nc.dram_tensor("wb", [V, D], f16, kind="ExternalInput").ap()
    idxf_d = nc.dram_tensor("idxf", [P, GF], i32, kind="ExternalInput").ap()
    idxb_d = nc.dram_tensor("idxb", [P, GB], i32, kind="ExternalInput").ap()
    out_d = nc.dram_tensor("out", [MLOC, NLOC], f16, kind="ExternalOutput").ap()

    n_copy = [0]
    # DVE also does the operand copies; ACT takes a larger strip share.
    STRIP_PAT = [1, 0, 1, 0, 1, 0, 1, 1, 0, 1, 0, 1, 0, 1, 1, 0]  # 1 = ACT (9/16)

    def strip_copy(dst, src):
        if STRIP_PAT[n_copy[0] % 16]:
            nc.scalar.copy(out=dst, in_=src)
        else:
            nc.vector.tensor_copy(out=dst, in_=src)
        n_copy[0] += 1

    with tile.TileContext(nc) as tc:
        with (
            tc.tile_pool(name="const", bufs=1) as const_pool,
            tc.tile_pool(name="idx", bufs=1) as idx_pool,
            tc.tile_pool(name="bg", bufs=16) as bg_pool,
            tc.tile_pool(name="ops", bufs=1) as ops_pool,
            tc.tile_pool(name="strip", bufs=8) as strip_pool,
            tc.tile_pool(name="tpsum", bufs=2, space="PSUM") as tpsum_pool,
            tc.tile_pool(name="mpsum", bufs=3, space="PSUM") as mpsum_pool,
        ):
            identity = const_pool.tile([P, P], f16)
            make_identity(nc, identity[:])

            idxf = idx_pool.tile([P, GF], i32, tag="idxf")
            idxb = idx_pool.tile([P, GB], i32, tag="idxb")
            nc.sync.dma_start(idxf[:], idxf_d[:])
            nc.sync.dma_start(idxb[:], idxb_d[:])

            fwdT = ops_pool.tile([D, MLOC], f16, tag="fwdT")
            bwdT = ops_pool.tile([D, NLOC], f16, tag="bwdT")

            def gather128(table, idx_tile, g):
                t = bg_pool.tile([P, D], f16, tag="bg")
                nc.gpsimd.indirect_dma_start(
                    out=t[:],
                    out_offset=None,
                    in_=table[:],
                    in_offset=bass.IndirectOffsetOnAxis(
                        ap=idx_tile[:, g:g + 1], axis=0
                    ),
                )
                return t

            def unit(table, idx_tile, g0, dstT, d0):
                """Gather 8 groups (g0..g0+7), transpose, copy into
                dstT[:, d0:d0+1024]."""
                pt = tpsum_pool.tile([D, JP], f16, tag="pt")
                for r in range(JP // P):
                    t = gather128(table, idx_tile, g0 + r)
                    nc.tensor.transpose(
                        out=pt[:, r * P:(r + 1) * P],
                        in_=t[:],
                        identity=identity[:],
                    )
                nc.vector.tensor_copy(out=dstT[:, d0:d0 + JP], in_=pt[:])

            def mm_strip(i, jp):
                """Row tile i x col strip jp: 2 matmuls, strip copy, DMA."""
                ps = mpsum_pool.tile([P, JP], f32, tag="ps")
                for q in range(JP // NJ):
                    j0 = jp * JP + q * NJ
                    nc.tensor.matmul(
                        out=ps[:, q * NJ:(q + 1) * NJ],
                        lhsT=fwdT[:, i * P:(i + 1) * P],
                        rhs=bwdT[:, j0:j0 + NJ],
                        start=True,
                        stop=True,
                    )
                strip = strip_pool.tile([P, JP], f16, tag="strip")
                strip_copy(strip[:], ps[:])
                nc.sync.dma_start(
                    out_d[i * P:(i + 1) * P, jp * JP:(jp + 1) * JP], strip[:]
                )

            # Interleaved program: gather/transpose units feed the per-engine
            # in-order streams ahead of the matmul batches that consume them.
            unit(wf, idxf, 0, fwdT, 0)        # fwd groups 0-7
            unit(wb, idxb, 0, bwdT, 0)        # bwd chunk 0 (cols 0-1023)
            for i in range(4):
                mm_strip(i, 0)
            unit(wf, idxf, 8, fwdT, JP)       # fwd groups 8-15
            for i in range(4, 8):
                mm_strip(i, 0)
            unit(wb, idxb, 8, bwdT, JP)       # bwd chunk 1
            for i in range(8, 16):
                mm_strip(i, 0)
            unit(wb, idxb, 16, bwdT, 2 * JP)  # bwd chunk 2
            for i in range(16):
                mm_strip(i, 1)
            unit(wb, idxb, 24, bwdT, 3 * JP)  # bwd chunk 3
            for i in range(16):
                mm_strip(i, 2)
            for i in range(16):
                mm_strip(i, 3)

    nc.compile()
    return nc


def _get_nc():
    if "nc" not in _CACHE:
        _CACHE["nc"] = _build_nc()
    return _CACHE["nc"]


def _ravel_clip(obs, act):
    o = np.clip(obs.astype(np.int64), 0, NUM_OBS - 1)
    a = np.clip(act.astype(np.int64), 0, NUM_ACT - 1)
    return (o * NUM_ACT + a).astype(np.int32)


def make_in_maps(observations, actions, future_observations, future_actions,
                 W_f, W_b):
    fwd_idx = _ravel_clip(np.asarray(observations), np.asarray(actions))
    bwd_idx = _ravel_clip(np.asarray(future_observations),
                          np.asarray(future_actions))
    wf = np.asarray(W_f, dtype=np.float16)
    wb = np.asarray(W_b, dtype=np.float16)
    in_maps = []
    for c in range(N_CORES):
        a, b = divmod(c, CB)
        # [p, g] = idx[g*128 + p]
        idxf = np.ascontiguousarray(
            fwd_idx[a * MLOC:(a + 1) * MLOC].reshape(GF, P).T
        )
        idxb = np.ascontiguousarray(
            bwd_idx[b * NLOC:(b + 1) * NLOC].reshape(GB, P).T
        )
        in_maps.append({"wf": wf, "wb": wb, "idxf": idxf, "idxb": idxb})
    return in_maps


def kernel(**inputs):
    from concourse.bass_utils import run_bass_kernel_spmd

    in_maps = make_in_maps(
        inputs["observations"], inputs["actions"],
        inputs["future_observations"], inputs["future_actions"],
        inputs["W_f"], inputs["W_b"],
    )
    res = run_bass_kernel_spmd(_get_nc(), in_maps, core_ids=list(range(N_CORES)))
    full = np.empty((B, B), dtype=np.float32)
    for c in range(N_CORES):
        a, b = divmod(c, CB)
        full[a * MLOC:(a + 1) * MLOC, b * NLOC:(b + 1) * NLOC] = (
            res.results[c]["out"].astype(np.float32)
        )
    return full
